# revision 21
# baseline (speedup 1.0000x reference)
"""Multi-head attention (B=4, S=2048, D=1024, H=16, dk=64) on 8 TRN2 cores.

Sharding: data-parallel over B (4 batches) x tensor-parallel over head
groups (2 groups of 8 heads).  Core c handles batch c//2 and head group
c%2: it computes Q/K/V with the 512-column slice of the projection
weights, runs attention for its 8 heads, and produces a partial output
projection through the matching 512-row slice of W_o.  The host sums the
two partials per batch and adds the constant bias term (bv @ Wo^T + bo).

v4: bf16 pipeline with fp8 DoubleRow scores.
  - All matmuls bf16 (1 cycle/row) except the scores matmul, which runs
    Q/K in fp8e4 DoubleRow (0.5 cycles/row): Q^T and K are stored as
    [64, 2, S] with the 128-wide e-contraction split into two 64-row
    sub-tiles that one DoubleRow matmul consumes at once.  fp8 on the
    scores path only perturbs softmax weights (~1.7% each), which
    averages out across 2048 keys; V/eps stay bf16.
  - The K bias is dropped entirely (softmax is invariant to per-query
    constants -- exact).  The Q bias folds into the PSUM->SBUF cast on
    the DVE.
  - x is loaded once (bf16) and stays resident; Q/K/V projections are
    hoisted, then attention runs as one ACT/PE pipeline per head with
    scores double-buffered through 4 PSUM banks and 4 banks of per-head
    output accumulators (ones column in V produces the softmax
    denominator for free).
  - Softmax has no max subtraction (scores are O(1)); denominators for
    all 4 q-blocks of a head are gathered into one [4, 512] tile so a
    single DVE reciprocal covers the head (its cost is free-size-bound).
"""

import sys

for _p in ("/opt/trn_rl_repo",):
    if _p not in sys.path:
        sys.path.insert(0, _p)

import numpy as np
import ml_dtypes
from contextlib import ExitStack

import concourse.bass as bass
import concourse.mybir as mybir
import concourse.tile as tile
from concourse import bacc
from concourse.bass_utils import run_bass_kernel_spmd

F32 = mybir.dt.float32
BF16 = mybir.dt.bfloat16
FP8 = mybir.dt.float8e4
AF = mybir.ActivationFunctionType
DR = mybir.MatmulPerfMode.DoubleRow

D, S = 1024, 2048   # d_model, seq len
E = 512             # per-core projection width (8 heads x 64)
H, DK = 8, 64       # heads per core, head dim
NB = D // 128       # contraction chunks (8)
SCALE = 0.125       # 1/sqrt(dk)


def build_bass(use_fp8_scores=False):
    nc = bacc.Bacc(
        "TRN2", target_bir_lowering=False, debug=False, num_devices=8
    )
    xT = nc.dram_tensor("xT", [D, S], BF16, kind="ExternalInput").ap()
    # wq/wk are pre-swizzled on the host to [p, et*dc*128] so the
    # per-e-tile DMA is a single contiguous 2KB-per-partition read
    wq = nc.dram_tensor("wq", [128, 4 * NB * 128], BF16, kind="ExternalInput").ap()
    wk = nc.dram_tensor("wk", [128, 4 * NB * 128], BF16, kind="ExternalInput").ap()
    wv = nc.dram_tensor("wv", [D, E], BF16, kind="ExternalInput").ap()
    wo = nc.dram_tensor("wo", [E, D], BF16, kind="ExternalInput").ap()
    bq = nc.dram_tensor("bq", [E], F32, kind="ExternalInput").ap()
    y = nc.dram_tensor("y", [S, D], BF16, kind="ExternalOutput").ap()

    qk_dt = FP8 if use_fp8_scores else BF16

    with ExitStack() as ctx:
        tc = ctx.enter_context(tile.TileContext(nc))
        const = ctx.enter_context(tc.tile_pool(name="const", bufs=1))
        wpool = ctx.enter_context(tc.tile_pool(name="wpool", bufs=8))
        res = ctx.enter_context(tc.tile_pool(name="res", bufs=1))
        epool = ctx.enter_context(tc.tile_pool(name="epool", bufs=6))
        ypool = ctx.enter_context(tc.tile_pool(name="ypool", bufs=2))
        bcpool = ctx.enter_context(tc.tile_pool(name="bcpool", bufs=2))
        rpool = ctx.enter_context(tc.tile_pool(name="rpool", bufs=2))
        ps_s = ctx.enter_context(tc.tile_pool(name="ps_s", bufs=2, space="PSUM"))
        ps_o = ctx.enter_context(tc.tile_pool(name="ps_o", bufs=4, space="PSUM"))

        # ---- constants ----
        bq_t = const.tile([128, 4], F32, tag="bq", name="bq_t")
        ones_f = const.tile([128, 8], BF16, tag="ones_f", name="ones_f")
        nc.sync.dma_start(bq_t[:, :], bq.rearrange("(j p) -> p j", p=128))
        nc.vector.memset(ones_f[:, :], 1.0)

        # ---- residents ----
        # x (whole input), Q^T / K per head in DoubleRow [64, 2, S] layout,
        # V with ones column, attn-out^T per e-tile
        xr = [
            res.tile([128, S], BF16, tag="x", bufs=NB, name=f"x{dc}")
            for dc in range(NB)
        ]
        # fp8 DoubleRow layout: [128, 2, S] -- sub-tile 0 carries the real
        # 128-row e-contraction, sub-tile 1 is zero (DoubleRow needs the
        # full 128 partitions; the zero sub-tile adds no cycles since
        # matmul time is moving-free-size bound)
        qk_shape = [128, 2, S] if use_fp8_scores else [128, S]
        q8 = [
            res.tile(qk_shape, qk_dt, tag="q8", bufs=4, name=f"q8_{et}")
            for et in range(4)
        ]
        k8 = [
            res.tile(qk_shape, qk_dt, tag="k8", bufs=8, name=f"k8_{h}")
            for h in range(H)
        ]
        vt = [
            res.tile([128, H, 65], BF16, tag="vt", bufs=16, name=f"vt{i}")
            for i in range(16)
        ]
        ao = [
            res.tile([128, S], BF16, tag="ao", bufs=4, name=f"ao{i}")
            for i in range(4)
        ]

        # zero the unused regions of the K/Q tiles (done once): head h
        # (hh = h%2) owns e-rows hh*64..hh*64+63 of sub-tile 0; everything
        # else contracts to zero
        for h in range(H):
            hh = h % 2
            if use_fp8_scores:
                nc.vector.memset(k8[h][:, 1, :], 0.0)
                nc.vector.memset(
                    k8[h][(1 - hh) * 64 : (1 - hh) * 64 + 64, 0, :], 0.0
                )
            else:
                nc.vector.memset(k8[h][(1 - hh) * 64 : (1 - hh) * 64 + 64, :], 0.0)
        if use_fp8_scores:
            for et in range(4):
                nc.vector.memset(q8[et][:, 1, :], 0.0)

        # ---- input DMAs (first-needed first) ----
        wq_t, wk_t = [], []

        def load_wqk(et):
            esl = slice(et * NB * 128, (et + 1) * NB * 128)
            wqt = wpool.tile([128, NB, 128], BF16, tag="wqk", bufs=8, name=f"wq{et}")
            nc.sync.dma_start(
                wqt[:, :, :], wq[:, esl].rearrange("p (dc e) -> p dc e", dc=NB)
            )
            wq_t.append(wqt)
            wkt = wpool.tile([128, NB, 128], BF16, tag="wqk", bufs=8, name=f"wk{et}")
            nc.sync.dma_start(
                wkt[:, :, :], wk[:, esl].rearrange("p (dc e) -> p dc e", dc=NB)
            )
            wk_t.append(wkt)

        # et0 weights and x first (the first projection matmuls need them);
        # the rest stream in behind
        load_wqk(0)
        for dc in range(NB):
            nc.sync.dma_start(xr[dc][:, :], xT[dc * 128 : (dc + 1) * 128, :])
        wv_t = []
        for dc in range(NB):
            wvt = wpool.tile([128, 512], BF16, tag="wv", bufs=8, name=f"wv{dc}")
            nc.sync.dma_start(wvt[:, :], wv[dc * 128 : (dc + 1) * 128, :])
            wv_t.append(wvt)
        for et in range(1, 4):
            load_wqk(et)
        wo_t = []
        for ec in range(4):
            wot = wpool.tile([128, 1024], BF16, tag="wo", bufs=4, name=f"wo{ec}")
            nc.sync.dma_start(wot[:, :], wo[ec * 128 : (ec + 1) * 128, :])
            wo_t.append(wot)

        # ---- Q/K projections (all e-tiles up front) ----
        # pp[e-tile 128, s 512] = sum_dc wq_t[:, dc, :]^T @ x[dc]; the
        # PSUM->SBUF cast re-lays [128, s] as [64, 2, s] (e = i*64 + p)
        for et in range(4):
            for sc in range(4):
                sl_ = slice(sc * 512, (sc + 1) * 512)
                for wt_, is_k in ((wq_t[et], False), (wk_t[et], True)):
                    pp = ps_s.tile([128, 512], F32, tag="s", name=f"pp{et}_{sc}_{is_k}")
                    for dc in range(NB):
                        nc.tensor.matmul(
                            pp[:, :],
                            (wt_[:, dc, :]),
                            (xr[dc][:, sl_]),
                            start=(dc == 0),
                            stop=(dc == NB - 1),
                        )
                    if not is_k and use_fp8_scores:
                        # fold the Q bias in during the cast (DVE)
                        nc.vector.tensor_scalar_add(
                            q8[et][:, 0, sl_], pp[:, :], bq_t[:, et : et + 1]
                        )
                        continue
                    for i in range(2):
                        pr = slice(i * 64, i * 64 + 64)
                        if not is_k:
                            nc.vector.tensor_scalar_add(
                                q8[et][pr, sl_], pp[pr, :], bq_t[pr, et : et + 1]
                            )
                        else:
                            kdst = (
                                k8[2 * et + i][pr, 0, sl_]
                                if use_fp8_scores
                                else k8[2 * et + i][pr, sl_]
                            )
                            nc.vector.tensor_copy(kdst, pp[pr, :])

        # ---- V projection (x stationary, Wv moving) ----
        for sc in range(4):
            for st in range(4):
                s_abs = sc * 4 + st
                vp = ps_s.tile([128, 512], F32, tag="s", name=f"vp{s_abs}")
                for dc in range(NB):
                    nc.tensor.matmul(
                        vp[:, :],
                        (xr[dc][:, s_abs * 128 : (s_abs + 1) * 128]),
                        (wv_t[dc][:, :]),
                        start=(dc == 0),
                        stop=(dc == NB - 1),
                    )
                nc.vector.tensor_copy(
                    vt[s_abs][:, :, 0:64], vp.rearrange("p (h d) -> p h d", h=H)
                )
                nc.vector.tensor_copy(
                    vt[s_abs][:, :, 64:65],
                    ones_f[:, 0:8].rearrange("p (h o) -> p h o", o=1),
                )

        # ---- attention (8 heads, ACT/PE software pipeline over k-tiles) ----
        for h in range(H):
            et, hh = divmod(h, 2)
            off = hh * 64
            o_ps = [
                ps_o.tile([65, 512], F32, tag="o", name=f"o{h}_{qc}")
                for qc in range(4)
            ]
            prev_eps = None
            for kt in range(17):
                eps = []
                if kt < 16:
                    for pr in range(2):
                        sp = ps_s.tile(
                            [128, 1024], F32, tag="s", name=f"sp{h}_{kt}_{pr}"
                        )
                        for half in range(2):
                            qc = 2 * pr + half
                            if use_fp8_scores:
                                k_ap = k8[h][:, :, kt * 128 : (kt + 1) * 128]
                                q_ap = q8[et][:, :, qc * 512 : (qc + 1) * 512]
                            else:
                                k_ap = k8[h][:, kt * 128 : (kt + 1) * 128]
                                q_ap = q8[et][:, qc * 512 : (qc + 1) * 512]
                            nc.tensor.matmul(
                                sp[:, half * 512 : (half + 1) * 512],
                                k_ap,
                                q_ap,
                                start=True,
                                stop=True,
                                perf_mode=DR if use_fp8_scores else None,
                            )
                        ep = epool.tile(
                            [128, 1024], BF16, tag="e", name=f"ep{h}_{kt}_{pr}"
                        )
                        nc.scalar.activation(ep[:, :], sp[:, :], AF.Exp, scale=SCALE)
                        eps.append(ep)
                if prev_eps is not None:
                    pk = kt - 1
                    for qc in range(4):
                        nc.tensor.matmul(
                            o_ps[qc][:, :],
                            (vt[pk][:, h, :]),
                            (prev_eps[qc // 2][:, (qc % 2) * 512 : (qc % 2 + 1) * 512]),
                            start=(pk == 0),
                            stop=(pk == 15),
                        )
                prev_eps = eps if kt < 16 else None
            # division: gather the 4 denominator rows, one reciprocal for
            # the whole head (DVE cost is free-size-bound), then
            # broadcast+multiply per q-block
            for qc in range(4):
                # copy numerator+denominator to SBUF first: releases the
                # PSUM bank for the next head immediately
                o_sb = bcpool.tile(
                    [65, 512], F32, tag="osb", bufs=4, name=f"ob{h}_{qc}"
                )
                nc.vector.tensor_copy(o_sb[:, :], o_ps[qc][:, :])
                recip = rpool.tile([1, 512], F32, tag="r", name=f"rc{h}_{qc}")
                nc.vector.reciprocal(recip[:, :], o_sb[64:65, :])
                # broadcast 1/denom to 64 partitions on the idle GpSimd
                bc_sb = bcpool.tile([64, 512], F32, tag="bc", name=f"bs{h}_{qc}")
                nc.gpsimd.partition_broadcast(bc_sb[:, :], recip[:, :])
                nc.vector.tensor_mul(
                    ao[et][off : off + 64, qc * 512 : (qc + 1) * 512],
                    o_sb[0:64, :],
                    bc_sb[:, :],
                )

        # ---- output projection (partial: this core's 512 e-rows of Wo) ----
        for qt in range(16):
            yps = [
                ps_s.tile([128, 512], F32, tag="s", name=f"yp{qt}_{oc}")
                for oc in range(2)
            ]
            for ec in range(4):
                for oc in range(2):
                    nc.tensor.matmul(
                        yps[oc][:, :],
                        (ao[ec][:, qt * 128 : (qt + 1) * 128]),
                        (wo_t[ec][:, oc * 512 : (oc + 1) * 512]),
                        start=(ec == 0),
                        stop=(ec == 3),
                    )
            ysb = ypool.tile([128, 1024], BF16, tag="y", name=f"ysb{qt}")
            for oc in range(2):
                nc.vector.tensor_copy(ysb[:, oc * 512 : (oc + 1) * 512], yps[oc][:, :])
            nc.sync.dma_start(y[qt * 128 : (qt + 1) * 128, :], ysb[:, :])

    nc.finalize()
    return nc


def _swizzle_wqk(W, sl):
    # [D, E_slice]^T laid out as [p, et, dc, ec] so each e-tile's weights
    # are one contiguous DMA: value at (p, et, dc, ec) = W.T[dc*128+p,
    # et*128+ec]
    wT = np.ascontiguousarray(W[sl, :].T)          # [1024 d, 512 e]
    w4 = wT.reshape(NB, 128, 4, 128)               # [dc, p, et, ec]
    w4 = np.ascontiguousarray(w4.transpose(1, 2, 0, 3))  # [p, et, dc, ec]
    return w4.reshape(128, 4 * NB * 128)


def make_in_maps(x, Wq, Wk, Wv, Wo, bq):
    bf = ml_dtypes.bfloat16
    in_maps = []
    for c in range(8):
        b, g = divmod(c, 2)
        sl = slice(g * E, (g + 1) * E)
        in_maps.append(
            {
                "xT": np.ascontiguousarray(x[b].T).astype(bf),
                "wq": _swizzle_wqk(Wq, sl).astype(bf),
                "wk": _swizzle_wqk(Wk, sl).astype(bf),
                "wv": np.ascontiguousarray(Wv[sl, :].T).astype(bf),
                "wo": np.ascontiguousarray(Wo[:, sl].T).astype(bf),
                "bq": np.ascontiguousarray(bq[sl], dtype=np.float32),
            }
        )
    return in_maps


_NC = None


def run(x, Wq, bq, Wk, bk, Wv, bv, Wo, bo, build_kwargs=None, **run_kwargs):
    global _NC
    x = np.asarray(x, dtype=np.float32)
    Wq, Wk, Wv, Wo = (np.asarray(a, dtype=np.float32) for a in (Wq, Wk, Wv, Wo))
    bq, bk, bv, bo = (np.asarray(a, dtype=np.float32) for a in (bq, bk, bv, bo))
    if _NC is None:
        _NC = build_bass(**(build_kwargs or {}))
    in_maps = make_in_maps(x, Wq, Wk, Wv, Wo, bq)
    try:
        res = run_bass_kernel_spmd(
            _NC, in_maps, core_ids=list(range(8)), **run_kwargs
        )
    except Exception:
        # One retry: a previously wedged device can fail the first attempt.
        res = run_bass_kernel_spmd(
            _NC, in_maps, core_ids=list(range(8)), **run_kwargs
        )
    ys = [np.asarray(r["y"], dtype=np.float32) for r in res.results]
    c_vec = (bv @ Wo.T + bo).astype(np.float32)  # constant bias fold
    out = np.stack([ys[2 * b] + ys[2 * b + 1] + c_vec for b in range(4)])
    return out.astype(np.float32), res


def kernel(x, Wq, bq, Wk, bk, Wv, bv, Wo, bo):
    out, _ = run(x, Wq, bq, Wk, bk, Wv, bv, Wo, bo)
    return out


# revision 22
# speedup vs baseline: 1.1615x; 1.1615x over previous
"""Multi-head attention (B=4, S=2048, D=1024, H=16, dk=64) on 8 TRN2 cores.

Sharding: data-parallel over B (4 batches) x tensor-parallel over head
groups (2 groups of 8 heads).  Core c handles batch c//2 and head group
c%2: it computes Q/K/V with the 512-column slice of the projection
weights, runs attention for its 8 heads, and produces a partial output
projection through the matching 512-row slice of W_o.  The host sums the
two partials per batch and adds the constant bias term (bv @ Wo^T + bo).

v4: bf16 pipeline with fp8 DoubleRow scores.
  - All matmuls bf16 (1 cycle/row) except the scores matmul, which runs
    Q/K in fp8e4 DoubleRow (0.5 cycles/row): Q^T and K are stored as
    [64, 2, S] with the 128-wide e-contraction split into two 64-row
    sub-tiles that one DoubleRow matmul consumes at once.  fp8 on the
    scores path only perturbs softmax weights (~1.7% each), which
    averages out across 2048 keys; V/eps stay bf16.
  - The K bias is dropped entirely (softmax is invariant to per-query
    constants -- exact).  The Q bias folds into the PSUM->SBUF cast on
    the DVE.
  - x is loaded once (bf16) and stays resident; Q/K/V projections are
    hoisted, then attention runs as one ACT/PE pipeline per head with
    scores double-buffered through 4 PSUM banks and 4 banks of per-head
    output accumulators (ones column in V produces the softmax
    denominator for free).
  - Softmax has no max subtraction (scores are O(1)); denominators for
    all 4 q-blocks of a head are gathered into one [4, 512] tile so a
    single DVE reciprocal covers the head (its cost is free-size-bound).
"""

import sys

for _p in ("/opt/trn_rl_repo",):
    if _p not in sys.path:
        sys.path.insert(0, _p)

import numpy as np
import ml_dtypes
from contextlib import ExitStack

import concourse.bass as bass
import concourse.mybir as mybir
import concourse.tile as tile
from concourse import bacc
from concourse.bass_utils import run_bass_kernel_spmd

F32 = mybir.dt.float32
BF16 = mybir.dt.bfloat16
FP8 = mybir.dt.float8e4
AF = mybir.ActivationFunctionType
DR = mybir.MatmulPerfMode.DoubleRow

D, S = 1024, 2048   # d_model, seq len
E = 512             # per-core projection width (8 heads x 64)
H, DK = 8, 64       # heads per core, head dim
NB = D // 128       # contraction chunks (8)
SCALE = 0.125       # 1/sqrt(dk)


def build_bass(use_fp8_scores=False):
    nc = bacc.Bacc(
        "TRN2", target_bir_lowering=False, debug=False, num_devices=8
    )
    xT = nc.dram_tensor("xT", [D, S], BF16, kind="ExternalInput").ap()
    # wq/wk are pre-swizzled on the host to [p, et*dc*128] so the
    # per-e-tile DMA is a single contiguous 2KB-per-partition read
    wq = nc.dram_tensor("wq", [128, 4 * NB * 128], BF16, kind="ExternalInput").ap()
    wk = nc.dram_tensor("wk", [128, 4 * NB * 128], BF16, kind="ExternalInput").ap()
    wv = nc.dram_tensor("wv", [D, E], BF16, kind="ExternalInput").ap()
    wo = nc.dram_tensor("wo", [E, D], BF16, kind="ExternalInput").ap()
    bq = nc.dram_tensor("bq", [E], F32, kind="ExternalInput").ap()
    y = nc.dram_tensor("y", [S, D], BF16, kind="ExternalOutput").ap()

    qk_dt = FP8 if use_fp8_scores else BF16

    with ExitStack() as ctx:
        tc = ctx.enter_context(tile.TileContext(nc))
        const = ctx.enter_context(tc.tile_pool(name="const", bufs=1))
        wpool = ctx.enter_context(tc.tile_pool(name="wpool", bufs=8))
        res = ctx.enter_context(tc.tile_pool(name="res", bufs=1))
        epool = ctx.enter_context(tc.tile_pool(name="epool", bufs=6))
        ypool = ctx.enter_context(tc.tile_pool(name="ypool", bufs=2))
        bcpool = ctx.enter_context(tc.tile_pool(name="bcpool", bufs=2))
        rpool = ctx.enter_context(tc.tile_pool(name="rpool", bufs=2))
        ps_s = ctx.enter_context(tc.tile_pool(name="ps_s", bufs=2, space="PSUM"))
        ps_o = ctx.enter_context(tc.tile_pool(name="ps_o", bufs=4, space="PSUM"))

        # ---- constants ----
        bq_t = const.tile([128, 4], F32, tag="bq", name="bq_t")
        ones_f = const.tile([128, 8], BF16, tag="ones_f", name="ones_f")
        nc.sync.dma_start(bq_t[:, :], bq.rearrange("(j p) -> p j", p=128))
        nc.vector.memset(ones_f[:, :], 1.0)

        # ---- residents ----
        # x (whole input), Q^T / K per head in DoubleRow [64, 2, S] layout,
        # V with ones column, attn-out^T per e-tile
        xr = [
            res.tile([128, S], BF16, tag="x", bufs=NB, name=f"x{dc}")
            for dc in range(NB)
        ]
        # fp8 DoubleRow layout: [128, 2, S] -- sub-tile 0 carries the real
        # 128-row e-contraction, sub-tile 1 is zero (DoubleRow needs the
        # full 128 partitions; the zero sub-tile adds no cycles since
        # matmul time is moving-free-size bound)
        qk_shape = [128, 2, S] if use_fp8_scores else [128, S]
        q8 = [
            res.tile(qk_shape, qk_dt, tag="q8", bufs=4, name=f"q8_{et}")
            for et in range(4)
        ]
        k8 = [
            res.tile(qk_shape, qk_dt, tag="k8", bufs=8, name=f"k8_{h}")
            for h in range(H)
        ]
        vt = [
            res.tile([128, H, 65], BF16, tag="vt", bufs=16, name=f"vt{i}")
            for i in range(16)
        ]
        ao = [
            res.tile([128, S], BF16, tag="ao", bufs=4, name=f"ao{i}")
            for i in range(4)
        ]

        # zero the unused regions of the K/Q tiles (done once): head h
        # (hh = h%2) owns e-rows hh*64..hh*64+63 of sub-tile 0; everything
        # else contracts to zero
        for h in range(H):
            hh = h % 2
            if use_fp8_scores:
                nc.vector.memset(k8[h][:, 1, :], 0.0)
                nc.vector.memset(
                    k8[h][(1 - hh) * 64 : (1 - hh) * 64 + 64, 0, :], 0.0
                )
            else:
                nc.vector.memset(k8[h][(1 - hh) * 64 : (1 - hh) * 64 + 64, :], 0.0)
        if use_fp8_scores:
            for et in range(4):
                nc.vector.memset(q8[et][:, 1, :], 0.0)

        # ---- input DMAs (first-needed first) ----
        wq_t, wk_t = [], []

        def load_wqk(et):
            esl = slice(et * NB * 128, (et + 1) * NB * 128)
            wqt = wpool.tile([128, NB, 128], BF16, tag="wqk", bufs=8, name=f"wq{et}")
            nc.sync.dma_start(
                wqt[:, :, :], wq[:, esl].rearrange("p (dc e) -> p dc e", dc=NB)
            )
            wq_t.append(wqt)
            wkt = wpool.tile([128, NB, 128], BF16, tag="wqk", bufs=8, name=f"wk{et}")
            nc.sync.dma_start(
                wkt[:, :, :], wk[:, esl].rearrange("p (dc e) -> p dc e", dc=NB)
            )
            wk_t.append(wkt)

        # et0 weights and x first (the first projection matmuls need them);
        # the rest stream in behind
        load_wqk(0)
        for dc in range(NB):
            nc.sync.dma_start(xr[dc][:, :], xT[dc * 128 : (dc + 1) * 128, :])
        wv_t = []
        for dc in range(NB):
            wvt = wpool.tile([128, 512], BF16, tag="wv", bufs=8, name=f"wv{dc}")
            nc.sync.dma_start(wvt[:, :], wv[dc * 128 : (dc + 1) * 128, :])
            wv_t.append(wvt)
        for et in range(1, 4):
            load_wqk(et)
        wo_t = []
        for ec in range(4):
            wot = wpool.tile([128, 1024], BF16, tag="wo", bufs=4, name=f"wo{ec}")
            nc.sync.dma_start(wot[:, :], wo[ec * 128 : (ec + 1) * 128, :])
            wo_t.append(wot)

        # ---- Q/K projections (all e-tiles up front) ----
        # pp[e-tile 128, s 512] = sum_dc wq_t[:, dc, :]^T @ x[dc]; the
        # PSUM->SBUF cast re-lays [128, s] as [64, 2, s] (e = i*64 + p)
        for et in range(4):
            for sc in range(4):
                sl_ = slice(sc * 512, (sc + 1) * 512)
                for wt_, is_k in ((wq_t[et], False), (wk_t[et], True)):
                    pp = ps_s.tile([128, 512], F32, tag="s", name=f"pp{et}_{sc}_{is_k}")
                    for dc in range(NB):
                        nc.tensor.matmul(
                            pp[:, :],
                            (wt_[:, dc, :]),
                            (xr[dc][:, sl_]),
                            start=(dc == 0),
                            stop=(dc == NB - 1),
                        )
                    if not is_k and use_fp8_scores:
                        # fold the Q bias in during the cast (DVE)
                        nc.vector.tensor_scalar_add(
                            q8[et][:, 0, sl_], pp[:, :], bq_t[:, et : et + 1]
                        )
                        continue
                    for i in range(2):
                        pr = slice(i * 64, i * 64 + 64)
                        if not is_k:
                            nc.vector.tensor_scalar_add(
                                q8[et][pr, sl_], pp[pr, :], bq_t[pr, et : et + 1]
                            )
                        else:
                            kdst = (
                                k8[2 * et + i][pr, 0, sl_]
                                if use_fp8_scores
                                else k8[2 * et + i][pr, sl_]
                            )
                            nc.vector.tensor_copy(kdst, pp[pr, :])

        # ---- V projection (x stationary, Wv moving) ----
        for sc in range(4):
            for st in range(4):
                s_abs = sc * 4 + st
                vp = ps_s.tile([128, 512], F32, tag="s", name=f"vp{s_abs}")
                for dc in range(NB):
                    nc.tensor.matmul(
                        vp[:, :],
                        (xr[dc][:, s_abs * 128 : (s_abs + 1) * 128]),
                        (wv_t[dc][:, :]),
                        start=(dc == 0),
                        stop=(dc == NB - 1),
                    )
                nc.vector.tensor_copy(
                    vt[s_abs][:, :, 0:64], vp.rearrange("p (h d) -> p h d", h=H)
                )
                nc.vector.tensor_copy(
                    vt[s_abs][:, :, 64:65],
                    ones_f[:, 0:8].rearrange("p (h o) -> p h o", o=1),
                )

        # ---- attention (8 heads, ACT/PE software pipeline over k-tiles) ----
        for h in range(H):
            et, hh = divmod(h, 2)
            off = hh * 64
            o_ps = [
                ps_o.tile([65, 512], F32, tag="o", name=f"o{h}_{qc}")
                for qc in range(4)
            ]
            prev_eps = None
            for kt in range(17):
                eps = []
                if kt < 16:
                    for pr in range(2):
                        sp = ps_s.tile(
                            [128, 1024], F32, tag="s", name=f"sp{h}_{kt}_{pr}"
                        )
                        for half in range(2):
                            qc = 2 * pr + half
                            if use_fp8_scores:
                                k_ap = k8[h][:, :, kt * 128 : (kt + 1) * 128]
                                q_ap = q8[et][:, :, qc * 512 : (qc + 1) * 512]
                            else:
                                k_ap = k8[h][:, kt * 128 : (kt + 1) * 128]
                                q_ap = q8[et][:, qc * 512 : (qc + 1) * 512]
                            nc.tensor.matmul(
                                sp[:, half * 512 : (half + 1) * 512],
                                k_ap,
                                q_ap,
                                start=True,
                                stop=True,
                                perf_mode=DR if use_fp8_scores else None,
                            )
                        ep = epool.tile(
                            [128, 1024], BF16, tag="e", name=f"ep{h}_{kt}_{pr}"
                        )
                        nc.scalar.activation(ep[:, :], sp[:, :], AF.Exp, scale=SCALE)
                        eps.append(ep)
                if prev_eps is not None:
                    pk = kt - 1
                    for qc in range(4):
                        nc.tensor.matmul(
                            o_ps[qc][:, :],
                            (vt[pk][:, h, :]),
                            (prev_eps[qc // 2][:, (qc % 2) * 512 : (qc % 2 + 1) * 512]),
                            start=(pk == 0),
                            stop=(pk == 15),
                        )
                prev_eps = eps if kt < 16 else None
            # all four PSUM->SBUF copies FIRST so every o_ps bank is
            # released before the next head's attnV needs them; the slow
            # reciprocals then run off the critical path on the DVE
            o_sbs = []
            for qc in range(4):
                o_sb = bcpool.tile(
                    [65, 512], F32, tag="osb", bufs=4, name=f"ob{h}_{qc}"
                )
                nc.vector.tensor_copy(o_sb[:, :], o_ps[qc][:, :])
                o_sbs.append(o_sb)
            for qc in range(4):
                recip = rpool.tile([1, 512], F32, tag="r", name=f"rc{h}_{qc}")
                nc.vector.reciprocal(recip[:, :], o_sbs[qc][64:65, :])
                # broadcast 1/denom to 64 partitions on the idle GpSimd
                bc_sb = bcpool.tile([64, 512], F32, tag="bc", name=f"bs{h}_{qc}")
                nc.gpsimd.partition_broadcast(bc_sb[:, :], recip[:, :])
                nc.vector.tensor_mul(
                    ao[et][off : off + 64, qc * 512 : (qc + 1) * 512],
                    o_sbs[qc][0:64, :],
                    bc_sb[:, :],
                )

        # ---- output projection (partial: this core's 512 e-rows of Wo) ----
        for qt in range(16):
            yps = [
                ps_s.tile([128, 512], F32, tag="s", name=f"yp{qt}_{oc}")
                for oc in range(2)
            ]
            for ec in range(4):
                for oc in range(2):
                    nc.tensor.matmul(
                        yps[oc][:, :],
                        (ao[ec][:, qt * 128 : (qt + 1) * 128]),
                        (wo_t[ec][:, oc * 512 : (oc + 1) * 512]),
                        start=(ec == 0),
                        stop=(ec == 3),
                    )
            ysb = ypool.tile([128, 1024], BF16, tag="y", name=f"ysb{qt}")
            for oc in range(2):
                nc.vector.tensor_copy(ysb[:, oc * 512 : (oc + 1) * 512], yps[oc][:, :])
            nc.sync.dma_start(y[qt * 128 : (qt + 1) * 128, :], ysb[:, :])

    nc.finalize()
    return nc


def _swizzle_wqk(W, sl):
    # [D, E_slice]^T laid out as [p, et, dc, ec] so each e-tile's weights
    # are one contiguous DMA: value at (p, et, dc, ec) = W.T[dc*128+p,
    # et*128+ec]
    wT = np.ascontiguousarray(W[sl, :].T)          # [1024 d, 512 e]
    w4 = wT.reshape(NB, 128, 4, 128)               # [dc, p, et, ec]
    w4 = np.ascontiguousarray(w4.transpose(1, 2, 0, 3))  # [p, et, dc, ec]
    return w4.reshape(128, 4 * NB * 128)


def make_in_maps(x, Wq, Wk, Wv, Wo, bq):
    bf = ml_dtypes.bfloat16
    in_maps = []
    for c in range(8):
        b, g = divmod(c, 2)
        sl = slice(g * E, (g + 1) * E)
        in_maps.append(
            {
                "xT": np.ascontiguousarray(x[b].T).astype(bf),
                "wq": _swizzle_wqk(Wq, sl).astype(bf),
                "wk": _swizzle_wqk(Wk, sl).astype(bf),
                "wv": np.ascontiguousarray(Wv[sl, :].T).astype(bf),
                "wo": np.ascontiguousarray(Wo[:, sl].T).astype(bf),
                "bq": np.ascontiguousarray(bq[sl], dtype=np.float32),
            }
        )
    return in_maps


_NC = None


def run(x, Wq, bq, Wk, bk, Wv, bv, Wo, bo, build_kwargs=None, **run_kwargs):
    global _NC
    x = np.asarray(x, dtype=np.float32)
    Wq, Wk, Wv, Wo = (np.asarray(a, dtype=np.float32) for a in (Wq, Wk, Wv, Wo))
    bq, bk, bv, bo = (np.asarray(a, dtype=np.float32) for a in (bq, bk, bv, bo))
    if _NC is None:
        _NC = build_bass(**(build_kwargs or {}))
    in_maps = make_in_maps(x, Wq, Wk, Wv, Wo, bq)
    try:
        res = run_bass_kernel_spmd(
            _NC, in_maps, core_ids=list(range(8)), **run_kwargs
        )
    except Exception:
        # One retry: a previously wedged device can fail the first attempt.
        res = run_bass_kernel_spmd(
            _NC, in_maps, core_ids=list(range(8)), **run_kwargs
        )
    ys = [np.asarray(r["y"], dtype=np.float32) for r in res.results]
    c_vec = (bv @ Wo.T + bo).astype(np.float32)  # constant bias fold
    out = np.stack([ys[2 * b] + ys[2 * b + 1] + c_vec for b in range(4)])
    return out.astype(np.float32), res


def kernel(x, Wq, bq, Wk, bk, Wv, bv, Wo, bo):
    out, _ = run(x, Wq, bq, Wk, bk, Wv, bv, Wo, bo)
    return out


# revision 24
# speedup vs baseline: 1.1648x; 1.0028x over previous
"""Multi-head attention (B=4, S=2048, D=1024, H=16, dk=64) on 8 TRN2 cores.

Sharding: data-parallel over B (4 batches) x tensor-parallel over head
groups (2 groups of 8 heads).  Core c handles batch c//2 and head group
c%2: it computes Q/K/V with the 512-column slice of the projection
weights, runs attention for its 8 heads, and produces a partial output
projection through the matching 512-row slice of W_o.  The host sums the
two partials per batch and adds the constant bias term (bv @ Wo^T + bo).

v4: bf16 pipeline with fp8 DoubleRow scores.
  - All matmuls bf16 (1 cycle/row) except the scores matmul, which runs
    Q/K in fp8e4 DoubleRow (0.5 cycles/row): Q^T and K are stored as
    [64, 2, S] with the 128-wide e-contraction split into two 64-row
    sub-tiles that one DoubleRow matmul consumes at once.  fp8 on the
    scores path only perturbs softmax weights (~1.7% each), which
    averages out across 2048 keys; V/eps stay bf16.
  - The K bias is dropped entirely (softmax is invariant to per-query
    constants -- exact).  The Q bias folds into the PSUM->SBUF cast on
    the DVE.
  - x is loaded once (bf16) and stays resident; Q/K/V projections are
    hoisted, then attention runs as one ACT/PE pipeline per head with
    scores double-buffered through 4 PSUM banks and 4 banks of per-head
    output accumulators (ones column in V produces the softmax
    denominator for free).
  - Softmax has no max subtraction (scores are O(1)); denominators for
    all 4 q-blocks of a head are gathered into one [4, 512] tile so a
    single DVE reciprocal covers the head (its cost is free-size-bound).
"""

import sys

for _p in ("/opt/trn_rl_repo",):
    if _p not in sys.path:
        sys.path.insert(0, _p)

import numpy as np
import ml_dtypes
from contextlib import ExitStack

import concourse.bass as bass
import concourse.mybir as mybir
import concourse.tile as tile
from concourse import bacc
from concourse.bass_utils import run_bass_kernel_spmd

F32 = mybir.dt.float32
BF16 = mybir.dt.bfloat16
FP8 = mybir.dt.float8e4
AF = mybir.ActivationFunctionType
DR = mybir.MatmulPerfMode.DoubleRow

D, S = 1024, 2048   # d_model, seq len
E = 512             # per-core projection width (8 heads x 64)
H, DK = 8, 64       # heads per core, head dim
NB = D // 128       # contraction chunks (8)
SCALE = 0.125       # 1/sqrt(dk)


def build_bass(use_fp8_scores=False):
    nc = bacc.Bacc(
        "TRN2", target_bir_lowering=False, debug=False, num_devices=8
    )
    xT = nc.dram_tensor("xT", [D, S], BF16, kind="ExternalInput").ap()
    # wq/wk are pre-swizzled on the host to [p, et*dc*128] so the
    # per-e-tile DMA is a single contiguous 2KB-per-partition read
    wq = nc.dram_tensor("wq", [128, 4 * NB * 128], BF16, kind="ExternalInput").ap()
    wk = nc.dram_tensor("wk", [128, 4 * NB * 128], BF16, kind="ExternalInput").ap()
    wv = nc.dram_tensor("wv", [D, E], BF16, kind="ExternalInput").ap()
    wo = nc.dram_tensor("wo", [E, D], BF16, kind="ExternalInput").ap()
    bq = nc.dram_tensor("bq", [E], F32, kind="ExternalInput").ap()
    y = nc.dram_tensor("y", [S, D], BF16, kind="ExternalOutput").ap()

    qk_dt = FP8 if use_fp8_scores else BF16

    with ExitStack() as ctx:
        tc = ctx.enter_context(tile.TileContext(nc))
        const = ctx.enter_context(tc.tile_pool(name="const", bufs=1))
        wpool = ctx.enter_context(tc.tile_pool(name="wpool", bufs=8))
        res = ctx.enter_context(tc.tile_pool(name="res", bufs=1))
        epool = ctx.enter_context(tc.tile_pool(name="epool", bufs=6))
        ypool = ctx.enter_context(tc.tile_pool(name="ypool", bufs=2))
        bcpool = ctx.enter_context(tc.tile_pool(name="bcpool", bufs=2))
        rpool = ctx.enter_context(tc.tile_pool(name="rpool", bufs=2))
        ps_s = ctx.enter_context(tc.tile_pool(name="ps_s", bufs=2, space="PSUM"))
        ps_o = ctx.enter_context(tc.tile_pool(name="ps_o", bufs=4, space="PSUM"))

        # ---- constants ----
        bq_t = const.tile([128, 4], F32, tag="bq", name="bq_t")
        ones_f = const.tile([128, 8], BF16, tag="ones_f", name="ones_f")
        nc.sync.dma_start(bq_t[:, :], bq.rearrange("(j p) -> p j", p=128))
        nc.vector.memset(ones_f[:, :], 1.0)

        # ---- residents ----
        # x (whole input), Q^T / K per head in DoubleRow [64, 2, S] layout,
        # V with ones column, attn-out^T per e-tile
        xr = [
            res.tile([128, S], BF16, tag="x", bufs=NB, name=f"x{dc}")
            for dc in range(NB)
        ]
        # fp8 DoubleRow layout: [128, 2, S] -- sub-tile 0 carries the real
        # 128-row e-contraction, sub-tile 1 is zero (DoubleRow needs the
        # full 128 partitions; the zero sub-tile adds no cycles since
        # matmul time is moving-free-size bound)
        qk_shape = [128, 2, S] if use_fp8_scores else [128, S]
        q8 = [
            res.tile(qk_shape, qk_dt, tag="q8", bufs=4, name=f"q8_{et}")
            for et in range(4)
        ]
        k8 = [
            res.tile(qk_shape, qk_dt, tag="k8", bufs=8, name=f"k8_{h}")
            for h in range(H)
        ]
        vt = [
            res.tile([128, H, 65], BF16, tag="vt", bufs=16, name=f"vt{i}")
            for i in range(16)
        ]
        ao = [
            res.tile([128, S], BF16, tag="ao", bufs=4, name=f"ao{i}")
            for i in range(4)
        ]

        # zero the unused regions of the K/Q tiles (done once): head h
        # (hh = h%2) owns e-rows hh*64..hh*64+63 of sub-tile 0; everything
        # else contracts to zero
        for h in range(H):
            hh = h % 2
            if use_fp8_scores:
                nc.vector.memset(k8[h][:, 1, :], 0.0)
                nc.vector.memset(
                    k8[h][(1 - hh) * 64 : (1 - hh) * 64 + 64, 0, :], 0.0
                )
            else:
                nc.vector.memset(k8[h][(1 - hh) * 64 : (1 - hh) * 64 + 64, :], 0.0)
        if use_fp8_scores:
            for et in range(4):
                nc.vector.memset(q8[et][:, 1, :], 0.0)

        # ---- input DMAs (first-needed first) ----
        wq_t, wk_t = [], []

        def load_wqk(et):
            esl = slice(et * NB * 128, (et + 1) * NB * 128)
            wqt = wpool.tile([128, NB, 128], BF16, tag="wqk", bufs=8, name=f"wq{et}")
            nc.sync.dma_start(
                wqt[:, :, :], wq[:, esl].rearrange("p (dc e) -> p dc e", dc=NB)
            )
            wq_t.append(wqt)
            wkt = wpool.tile([128, NB, 128], BF16, tag="wqk", bufs=8, name=f"wk{et}")
            nc.sync.dma_start(
                wkt[:, :, :], wk[:, esl].rearrange("p (dc e) -> p dc e", dc=NB)
            )
            wk_t.append(wkt)

        # et0 weights and x first (the first projection matmuls need them);
        # the rest stream in behind
        load_wqk(0)
        for dc in range(NB):
            nc.sync.dma_start(xr[dc][:, :], xT[dc * 128 : (dc + 1) * 128, :])
        wv_t = []
        for dc in range(NB):
            wvt = wpool.tile([128, 512], BF16, tag="wv", bufs=8, name=f"wv{dc}")
            nc.sync.dma_start(wvt[:, :], wv[dc * 128 : (dc + 1) * 128, :])
            wv_t.append(wvt)
        for et in range(1, 4):
            load_wqk(et)
        wo_t = []
        for ec in range(4):
            wot = wpool.tile([128, 1024], BF16, tag="wo", bufs=4, name=f"wo{ec}")
            nc.sync.dma_start(wot[:, :], wo[ec * 128 : (ec + 1) * 128, :])
            wo_t.append(wot)

        # ---- Q/K projections (all e-tiles up front) ----
        # pp[e-tile 128, s 512] = sum_dc wq_t[:, dc, :]^T @ x[dc]; the
        # PSUM->SBUF cast re-lays [128, s] as [64, 2, s] (e = i*64 + p)
        for et in range(4):
            for sc in range(4):
                sl_ = slice(sc * 512, (sc + 1) * 512)
                for wt_, is_k in ((wq_t[et], False), (wk_t[et], True)):
                    pp = ps_s.tile([128, 512], F32, tag="s", name=f"pp{et}_{sc}_{is_k}")
                    for dc in range(NB):
                        nc.tensor.matmul(
                            pp[:, :],
                            (wt_[:, dc, :]),
                            (xr[dc][:, sl_]),
                            start=(dc == 0),
                            stop=(dc == NB - 1),
                        )
                    if not is_k and use_fp8_scores:
                        # fold the Q bias in during the cast (DVE)
                        nc.vector.tensor_scalar_add(
                            q8[et][:, 0, sl_], pp[:, :], bq_t[:, et : et + 1]
                        )
                        continue
                    for i in range(2):
                        pr = slice(i * 64, i * 64 + 64)
                        if not is_k:
                            nc.vector.tensor_scalar_add(
                                q8[et][pr, sl_], pp[pr, :], bq_t[pr, et : et + 1]
                            )
                        else:
                            kdst = (
                                k8[2 * et + i][pr, 0, sl_]
                                if use_fp8_scores
                                else k8[2 * et + i][pr, sl_]
                            )
                            nc.vector.tensor_copy(kdst, pp[pr, :])

        # ---- V projection (x stationary, Wv moving) ----
        for sc in range(4):
            for st in range(4):
                s_abs = sc * 4 + st
                vp = ps_s.tile([128, 512], F32, tag="s", name=f"vp{s_abs}")
                for dc in range(NB):
                    nc.tensor.matmul(
                        vp[:, :],
                        (xr[dc][:, s_abs * 128 : (s_abs + 1) * 128]),
                        (wv_t[dc][:, :]),
                        start=(dc == 0),
                        stop=(dc == NB - 1),
                    )
                nc.vector.tensor_copy(
                    vt[s_abs][:, :, 0:64], vp.rearrange("p (h d) -> p h d", h=H)
                )
                nc.vector.tensor_copy(
                    vt[s_abs][:, :, 64:65],
                    ones_f[:, 0:8].rearrange("p (h o) -> p h o", o=1),
                )

        # ---- attention (8 heads, ACT/PE software pipeline over k-tiles) ----
        def _div_tail(h, et, off, qc, o_sb):
            recip = rpool.tile([1, 512], F32, tag="r", name=f"rc{h}_{qc}")
            nc.vector.reciprocal(recip[:, :], o_sb[64:65, :])
            # broadcast 1/denom to 64 partitions on the idle GpSimd
            bc_sb = bcpool.tile([64, 512], F32, tag="bc", name=f"bs{h}_{qc}")
            nc.gpsimd.partition_broadcast(bc_sb[:, :], recip[:, :])
            nc.vector.tensor_mul(
                ao[et][off : off + 64, qc * 512 : (qc + 1) * 512],
                o_sb[0:64, :],
                bc_sb[:, :],
            )

        for h in range(H):
            et, hh = divmod(h, 2)
            off = hh * 64
            o_ps = [
                ps_o.tile([65, 512], F32, tag="o", name=f"o{h}_{qc}")
                for qc in range(4)
            ]
            prev_eps = None
            for kt in range(17):
                eps = []
                if kt < 16:
                    for pr in range(2):
                        sp = ps_s.tile(
                            [128, 1024], F32, tag="s", name=f"sp{h}_{kt}_{pr}"
                        )
                        for half in range(2):
                            qc = 2 * pr + half
                            if use_fp8_scores:
                                k_ap = k8[h][:, :, kt * 128 : (kt + 1) * 128]
                                q_ap = q8[et][:, :, qc * 512 : (qc + 1) * 512]
                            else:
                                k_ap = k8[h][:, kt * 128 : (kt + 1) * 128]
                                q_ap = q8[et][:, qc * 512 : (qc + 1) * 512]
                            nc.tensor.matmul(
                                sp[:, half * 512 : (half + 1) * 512],
                                k_ap,
                                q_ap,
                                start=True,
                                stop=True,
                                perf_mode=DR if use_fp8_scores else None,
                            )
                        ep = epool.tile(
                            [128, 1024], BF16, tag="e", name=f"ep{h}_{kt}_{pr}"
                        )
                        nc.scalar.activation(ep[:, :], sp[:, :], AF.Exp, scale=SCALE)
                        eps.append(ep)
                if prev_eps is not None:
                    pk = kt - 1
                    for qc in range(4):
                        nc.tensor.matmul(
                            o_ps[qc][:, :],
                            (vt[pk][:, h, :]),
                            (prev_eps[qc // 2][:, (qc % 2) * 512 : (qc % 2 + 1) * 512]),
                            start=(pk == 0),
                            stop=(pk == 15),
                        )
                prev_eps = eps if kt < 16 else None
            # Mid-run heads: all four PSUM->SBUF copies FIRST so every o_ps
            # bank is released before the next head's attnV needs them; the
            # slow reciprocals then run off the critical path on the DVE.
            # Last head: chain per-qc instead, minimizing latency to the
            # first ao write (which gates the output projection).
            o_sbs = []
            for qc in range(4):
                o_sb = bcpool.tile(
                    [65, 512], F32, tag="osb", bufs=4, name=f"ob{h}_{qc}"
                )
                nc.vector.tensor_copy(o_sb[:, :], o_ps[qc][:, :])
                o_sbs.append(o_sb)
                if h < H - 1:
                    continue
                _div_tail(h, et, off, qc, o_sbs[qc])
            if h < H - 1:
                for qc in range(4):
                    _div_tail(h, et, off, qc, o_sbs[qc])

        # ---- output projection (partial: this core's 512 e-rows of Wo) ----
        for qt in range(16):
            yps = [
                ps_s.tile([128, 512], F32, tag="s", name=f"yp{qt}_{oc}")
                for oc in range(2)
            ]
            for ec in range(4):
                for oc in range(2):
                    nc.tensor.matmul(
                        yps[oc][:, :],
                        (ao[ec][:, qt * 128 : (qt + 1) * 128]),
                        (wo_t[ec][:, oc * 512 : (oc + 1) * 512]),
                        start=(ec == 0),
                        stop=(ec == 3),
                    )
            ysb = ypool.tile([128, 1024], BF16, tag="y", name=f"ysb{qt}")
            for oc in range(2):
                nc.vector.tensor_copy(ysb[:, oc * 512 : (oc + 1) * 512], yps[oc][:, :])
            nc.sync.dma_start(y[qt * 128 : (qt + 1) * 128, :], ysb[:, :])

    nc.finalize()
    return nc


def _swizzle_wqk(W, sl):
    # [D, E_slice]^T laid out as [p, et, dc, ec] so each e-tile's weights
    # are one contiguous DMA: value at (p, et, dc, ec) = W.T[dc*128+p,
    # et*128+ec]
    wT = np.ascontiguousarray(W[sl, :].T)          # [1024 d, 512 e]
    w4 = wT.reshape(NB, 128, 4, 128)               # [dc, p, et, ec]
    w4 = np.ascontiguousarray(w4.transpose(1, 2, 0, 3))  # [p, et, dc, ec]
    return w4.reshape(128, 4 * NB * 128)


def make_in_maps(x, Wq, Wk, Wv, Wo, bq):
    bf = ml_dtypes.bfloat16
    in_maps = []
    for c in range(8):
        b, g = divmod(c, 2)
        sl = slice(g * E, (g + 1) * E)
        in_maps.append(
            {
                "xT": np.ascontiguousarray(x[b].T).astype(bf),
                "wq": _swizzle_wqk(Wq, sl).astype(bf),
                "wk": _swizzle_wqk(Wk, sl).astype(bf),
                "wv": np.ascontiguousarray(Wv[sl, :].T).astype(bf),
                "wo": np.ascontiguousarray(Wo[:, sl].T).astype(bf),
                "bq": np.ascontiguousarray(bq[sl], dtype=np.float32),
            }
        )
    return in_maps


_NC = None


def run(x, Wq, bq, Wk, bk, Wv, bv, Wo, bo, build_kwargs=None, **run_kwargs):
    global _NC
    x = np.asarray(x, dtype=np.float32)
    Wq, Wk, Wv, Wo = (np.asarray(a, dtype=np.float32) for a in (Wq, Wk, Wv, Wo))
    bq, bk, bv, bo = (np.asarray(a, dtype=np.float32) for a in (bq, bk, bv, bo))
    if _NC is None:
        _NC = build_bass(**(build_kwargs or {}))
    in_maps = make_in_maps(x, Wq, Wk, Wv, Wo, bq)
    try:
        res = run_bass_kernel_spmd(
            _NC, in_maps, core_ids=list(range(8)), **run_kwargs
        )
    except Exception:
        # One retry: a previously wedged device can fail the first attempt.
        res = run_bass_kernel_spmd(
            _NC, in_maps, core_ids=list(range(8)), **run_kwargs
        )
    ys = [np.asarray(r["y"], dtype=np.float32) for r in res.results]
    c_vec = (bv @ Wo.T + bo).astype(np.float32)  # constant bias fold
    out = np.stack([ys[2 * b] + ys[2 * b + 1] + c_vec for b in range(4)])
    return out.astype(np.float32), res


def kernel(x, Wq, bq, Wk, bk, Wv, bv, Wo, bo):
    out, _ = run(x, Wq, bq, Wk, bk, Wv, bv, Wo, bo)
    return out
